# revision 1
# baseline (speedup 1.0000x reference)
"""AKT (attentive knowledge tracing) forward pass on 8 TRN2 NeuronCores.

Sharding: batch b = core//2 across 4 core-pairs; within a pair, the 8 heads
of each of the 3 MHA blocks are split 4+4 (core%2).  Two pairwise AllReduces
merge the head-partial wO outputs (after mha1+mha2, and after mha3).

Device-side math per core (B=4, S=512, C=256, D=256, H=8, P=5000):
  concept = Qm[clip(item-1,0)] * valid        (dma_gather + diag-mask transpose)
  x = cmu @ d_embed + concept @ c_embed ;  y = r_embed[correct]*cn + cmu @ f_embed
  3x monotonic attention (AKT):
    scores = qx M qx^T   (M = wQ wK^T/sqrt(D) folded on host)
    P1 = masked softmax(scores);  decay = 1 - cumsum(P1)
    s = scores * exp(-theta^2 * decay * (t_i - t_j))
    A = masked softmax(s);  out += A vx wU_h   (wU = wV wO_h folded on host)
  pred = sigmoid([out3(shifted), x_hat] @ Wd + bd)

All matmuls run as bf16 or float32r (full-rate fp32); the softmax/decay
pipeline stays fp32 on DVE/ACT.  cumsum uses the HW prefix-scan
(tensor_tensor_scan); the A^T needed by the AV matmul is produced with DMA
transposes (bf16).
"""

import os
import numpy as np
import ml_dtypes

import concourse.bass as bass
import concourse.mybir as mybir
from concourse import bacc, tile
from concourse.bass_utils import run_bass_kernel_spmd

F32 = mybir.dt.float32
F32R = mybir.dt.float32r
BF16 = mybir.dt.bfloat16
I16 = mybir.dt.int16
AF = mybir.ActivationFunctionType
OP = mybir.AluOpType

B, S, P, C, D, H = 4, 512, 5000, 256, 256, 8
NB = S // 128          # 4 row blocks
NC_ = C // 128         # 2 chunks of C
ND = D // 128          # 2 chunks of D
HPC = H // 2           # heads per core
N_CORES = 8
NEG = -30000.0


def tctile(tc, shape, dt, name, _frees=[]):
    t, free = tc.tile(shape, dt, name=f"{name}_r{tctile.sfx}")
    tctile.frees.append(free)
    return t


def r32(ap):
    return ap.bitcast(F32R)


def build_kernel(debug=False, reps=1):
    nc = bacc.Bacc(None, target_bir_lowering=False, debug=False, num_devices=N_CORES)

    # ---------------- DRAM parameters (per-core shards, host-prepped) -------
    dp = lambda name, shape, dt: nc.declare_dram_parameter(name, shape, dt, isOutput=False)
    qm_d = dp("qm", [P + 1, C], F32)
    idx_d = dp("idx16", [128, S // 16], I16)
    tsa_d = dp("tsa", [2, S], F32)          # rows: ts, ones
    tsb_d = dp("tsb", [2, S], F32)          # rows: ones, -ts
    corr_d = dp("corr", [1, S], BF16)       # correct as bf16
    i128_d = dp("i128", [128, 128], F32)
    i128b_d = dp("i128b", [128, 128], BF16)
    maskns_d = dp("maskns", [128, 128], BF16)
    masks_d = dp("masks", [128, 128], BF16)
    onescol_d = dp("onescol", [128, 1], BF16)
    cemb_d = dp("cembed2", [128, NC_ * D], BF16)
    femb_d = dp("fembed2", [128, NC_ * D], BF16)
    r0_d = dp("r0", [1, D], BF16)
    dr_d = dp("dr", [1, D], BF16)
    m_d = {p: dp(p + "_m", [HPC, 128, ND * D], BF16) for p in "qkr"}
    wu_d = {p: dp(p + "_wu", [HPC, 128, ND * D], BF16) for p in "qkr"}
    th2_d = dp("th2s", [1, 3], F32)         # theta^2 for q, k, r
    wd_d = dp("wd2", [128, 2 * ND], BF16)    # cols: wd_o chunks, then wd_x chunks
    bd_d = dp("bd", [1, 1], F32)

    out_d = nc.declare_dram_parameter("out", [1, S], F32, isOutput=True)
    dbg_d = {}
    if debug:
        for name in ("xT", "yT"):
            dbg_d[name] = nc.declare_dram_parameter("dbg_" + name, [D, S], F32, isOutput=True)
        for name in ("xhatT", "yhatT"):
            dbg_d[name] = nc.declare_dram_parameter("dbg_" + name, [D, S], BF16, isOutput=True)
        dbg_d["crw"] = nc.declare_dram_parameter("dbg_crw", [128, NB * C], F32, isOutput=True)
        dbg_d["cT"] = nc.declare_dram_parameter("dbg_cT", [C, S], BF16, isOutput=True)
        dbg_d["rhs2"] = nc.declare_dram_parameter("dbg_rhs2", [2, S], BF16, isOutput=True)
        dbg_d["dt"] = nc.declare_dram_parameter("dbg_dt", [S, S], BF16, isOutput=True)

    from contextlib import ExitStack
    tctile.frees = []
    tctile.sfx = 0
    with tile.TileContext(nc) as tc, ExitStack() as es:
        # ---------------- pools ------------------------------------------------
        pp_acc = es.enter_context(tc.tile_pool(name="pp_acc", bufs=4, space="PSUM"))
        pp_sc = es.enter_context(tc.tile_pool(name="pp_sc", bufs=2, space="PSUM"))
        pp_gt = es.enter_context(tc.tile_pool(name="pp_gt", bufs=2, space="PSUM"))
        wk = es.enter_context(tc.tile_pool(name="wk", bufs=6))
        wpool = es.enter_context(tc.tile_pool(name="wpool", bufs=5))
        e2p = es.enter_context(tc.tile_pool(name="e2p", bufs=4))
        dram = es.enter_context(tc.tile_pool(name="dram", bufs=2, space="DRAM"))

        for rep in range(reps):
            tctile.sfx = rep
            rep_free_start = len(tctile.frees)
            # ---------------- persistent SBUF --------------------------------------
            tsa = tctile(tc, [2, S], F32, name="tsa")
            tsb = tctile(tc, [2, S], F32, name="tsb")
            corr = tctile(tc, [1, S], BF16, name="corr")
            i128 = tctile(tc, [128, 128], F32, name="i128")
            i128b = tctile(tc, [128, 128], BF16, name="i128b")
            maskns = tctile(tc, [128, 128], BF16, name="maskns")
            masks = tctile(tc, [128, 128], BF16, name="masks")
            onescol = tctile(tc, [128, 1], BF16, name="onescol")
            cemb = tctile(tc, [128, NC_ * D], BF16, name="cemb")
            femb = tctile(tc, [128, NC_ * D], BF16, name="femb")
            r0 = tctile(tc, [1, D], BF16, name="r0")
            drm = tctile(tc, [1, D], BF16, name="drm")
            th2s = tctile(tc, [1, 3], F32, name="th2s")
            wd2 = tctile(tc, [128, 2 * ND], BF16, name="wd2")
            bd = tctile(tc, [1, 1], F32, name="bd")
            idx16 = tctile(tc, [128, S // 16], I16, name="idx16")

            dt_sb = [tctile(tc, [128, S], BF16, name=f"dt{ib}") for ib in range(NB)]
            xT_bf = [tctile(tc, [128, S], BF16, name=f"xTbf{a}") for a in range(ND)]
            yT_bf = [tctile(tc, [128, S], BF16, name=f"yTbf{a}") for a in range(ND)]
            xhT_bf = [tctile(tc, [128, S], BF16, name=f"xhTbf{a}") for a in range(ND)]
            yhT_bf = [tctile(tc, [128, S], BF16, name=f"yhTbf{a}") for a in range(ND)]
            xpart = [tctile(tc, [128, S], BF16, name=f"xpart{a}") for a in range(ND)]
            ypart = [tctile(tc, [128, S], BF16, name=f"ypart{a}") for a in range(ND)]
            opart = [tctile(tc, [128, S], BF16, name=f"opart{a}") for a in range(ND)]

            # ---------------- load constants/weights --------------------------------
            for sb, dr in ((tsa, tsa_d), (tsb, tsb_d), (corr, corr_d),
                           (i128, i128_d)):
                nc.sync.dma_start(sb[:], dr[:])
            nc.sync.dma_start(th2s[:], th2_d[:])
            nc.sync.dma_start(idx16[:], idx_d[:])

            # ---------------- embedding ---------------------------------------------
            crw = tctile(tc, [128, NB, C], F32, name="crw")
            gsem_cm = nc.semaphore(f"gather_sem_r{tctile.sfx}", num=240 - tctile.sfx)
            gsem = gsem_cm.__enter__()
            tctile.frees.append(lambda: gsem_cm.__exit__(None, None, None))
            with tc.tile_critical():
                nc.gpsimd.dma_gather(crw[:], qm_d[:], idx16[:], S, S, C).then_inc(gsem, 16)
                nc.gpsimd.wait_ge(gsem, 16)

            if debug:
                nc.sync.dma_start(dbg_d["crw"][:], crw[:].rearrange("p a c -> p (a c)"))
            for sb, dr in ((i128b, i128b_d), (maskns, maskns_d), (masks, masks_d),
                           (onescol, onescol_d), (cemb, cemb_d),
                           (femb, femb_d), (r0, r0_d), (drm, dr_d),
                           (wd2, wd_d), (bd, bd_d)):
                nc.gpsimd.dma_start(sb[:], dr[:])
            conceptT = [tctile(tc, [128, S], BF16, name=f"cT{a}") for a in range(NC_)]
            for a in range(NC_):
                ct_ps = pp_gt.tile([128, S], F32, name="ct_ps", tag="gt")
                for g in range(NB):
                    nc.tensor.transpose(ct_ps[:, 128 * g:128 * (g + 1)],
                                        crw[:, g, 128 * a:128 * (a + 1)], i128[:])
                nc.scalar.copy(conceptT[a][:], ct_ps[:])

            if debug:
                for a in range(NC_):
                    nc.sync.dma_start(dbg_d["cT"][128 * a:128 * (a + 1), :], conceptT[a][:])
            # cn = ones^T @ conceptT  -> [1, S]
            cn_ps = pp_sc.tile([1, S], F32, name="cn_ps", tag="sc")
            for a in range(NC_):
                nc.tensor.matmul(cn_ps[:], onescol[:], conceptT[a][:],
                                 start=(a == 0), stop=(a == NC_ - 1))
            cn_sb = tctile(tc, [1, S], BF16, name="cn_sb")
            ccn_sb = tctile(tc, [1, S], BF16, name="ccn_sb")
            nc.scalar.copy(cn_sb[:], cn_ps[:])
            nc.vector.tensor_mul(ccn_sb[:], cn_sb[:], corr[:])

            if debug:
                nc.sync.dma_start(dbg_d["rhs2"][0:1, :], cn_sb[:])
                nc.sync.dma_start(dbg_d["rhs2"][1:2, :], ccn_sb[:])
            # xT = c_embed^T conceptT + d_embed^T cmuT ; yT = f_embed^T cmuT + r01^T rhs2
            for ec in range(ND):
                x_ps = pp_acc.tile([128, S], F32, name="x_ps", tag="acc")
                for a in range(NC_):
                    nc.tensor.matmul(x_ps[:], cemb[:, a * D + 128 * ec: a * D + 128 * (ec + 1)],
                                     conceptT[a][:], start=(a == 0), stop=(a == NC_ - 1))
                nc.scalar.copy(xT_bf[ec][:], x_ps[:])
                if debug:
                    dbgx = tctile(tc, [128, S], F32, name=f"dbgx{ec}")
                    nc.scalar.copy(dbgx[:], x_ps[:])
                    nc.sync.dma_start(dbg_d["xT"][128 * ec:128 * (ec + 1), :], dbgx[:])

                y_ps = pp_acc.tile([128, S], F32, name="y_ps", tag="acc")
                for a in range(NC_):
                    nc.tensor.matmul(y_ps[:], femb[:, a * D + 128 * ec: a * D + 128 * (ec + 1)],
                                     conceptT[a][:], start=(a == 0), stop=False)
                nc.tensor.matmul(y_ps[:], r0[:, 128 * ec:128 * (ec + 1)], cn_sb[:],
                                 start=False, stop=False)
                nc.tensor.matmul(y_ps[:], drm[:, 128 * ec:128 * (ec + 1)], ccn_sb[:],
                                 start=False, stop=True)
                nc.scalar.copy(yT_bf[ec][:], y_ps[:])
                if debug:
                    dbgy = tctile(tc, [128, S], F32, name=f"dbgy{ec}")
                    nc.scalar.copy(dbgy[:], y_ps[:])
                    nc.sync.dma_start(dbg_d["yT"][128 * ec:128 * (ec + 1), :], dbgy[:])


            # dt[i, j] = t_i - t_j, per row-block
            for ib in range(NB):
                dt_ps = pp_sc.tile([128, S], F32, name="dt_ps", tag="sc")
                nc.tensor.matmul(dt_ps[:], tsa[:, 128 * ib:128 * (ib + 1)],
                                 tsb[:], start=True, stop=True)
                nc.scalar.copy(dt_sb[ib][:], dt_ps[:])

            if debug:
                for ib in range(NB):
                    nc.sync.dma_start(dbg_d["dt"][128 * ib:128 * (ib + 1), :], dt_sb[ib][:])
            # ---------------- one MHA phase ----------------------------------------
            def mha_begin(p, idx, strict):
                mask = masks if strict else maskns
                # theta^2 broadcast to [128,1]
                t2_ps = pp_sc.tile([128, 1], F32, name="t2_ps", tag="sc")
                nc.tensor.matmul(t2_ps[:], tsb[0:1, 0:128], th2s[:, idx:idx + 1],
                                 start=True, stop=True)
                th2b = tctile(tc, [128, 1], F32, name=f"th2b_{p}")
                nc.scalar.copy(th2b[:], t2_ps[:])
                o_ps = [pp_acc.tile([128, S], F32, name=f"o_ps{p}{ec}", tag="acc")
                        for ec in range(ND)]
                return dict(p=p, mask=mask, th2b=th2b, o_ps=o_ps, strict=strict)

            def mha_head(st, h, qxT, vxT):
                """qxT/vxT: [ND] bf16 [128,S] transposed activations."""
                p, mask, strict = st["p"], st["mask"], st["strict"]
                th2b, o_ps = st["th2b"], st["o_ps"]
                m_sb = wpool.tile([128, ND * D], BF16, name="m_sb", tag="m_sb")
                wu_sb = wpool.tile([128, ND * D], BF16, name="wu_sb", tag="wu_sb")
                nc.gpsimd.dma_start(m_sb[:], m_d[p][h])
                nc.gpsimd.dma_start(wu_sb[:], wu_d[p][h])

                # WV[j, e] = sum_d vx[j, d] wU[d, e]  (off the critical path)
                wv_sb = [wpool.tile([128, S], BF16, name=f"wv{q_}", tag=f"wv{q_}")
                         for q_ in range(2)]
                for half in range(2):
                    wv_ps = pp_gt.tile([128, S], F32, name="wv_ps", tag="gt")
                    for j2 in range(2):
                        jb = 2 * half + j2
                        for a in range(ND):
                            nc.tensor.matmul(
                                wv_ps[:, 256 * j2:256 * (j2 + 1)],
                                vxT[a][:, 128 * jb:128 * (jb + 1)],
                                wu_sb[:, a * D:(a + 1) * D],
                                start=(a == 0), stop=(a == ND - 1))
                    nc.scalar.copy(wv_sb[half][:], wv_ps[:])

                # G^T = M^T qx^T
                gt_sb = [wpool.tile([128, S], BF16, name=f"gt{ec}", tag=f"gt{ec}")
                         for ec in range(ND)]
                for ec in range(ND):
                    gt_ps = pp_gt.tile([128, S], F32, name="gt_ps", tag="gt")
                    for a in range(ND):
                        nc.tensor.matmul(gt_ps[:],
                                         m_sb[:, a * D + 128 * ec: a * D + 128 * (ec + 1)],
                                         qxT[a][:], start=(a == 0), stop=(a == ND - 1))
                    nc.scalar.copy(gt_sb[ec][:], gt_ps[:])

                e2t = e2p.tile([128, NB * NB, 128], BF16, name="e2t", tag="e2t")
                a_full = e2p.tile([128, NB, S], BF16, name="a_full", tag="a_full")
                if os.environ.get("AKT_SIM"):
                    nc.gpsimd.memset(a_full[:], 0)
                for ib in range(NB):
                    J = 128 * (ib + 1)
                    # scores (PSUM) + additive mask on the diagonal block
                    sc_ps = pp_sc.tile([128, S], F32, name="sc_ps", tag="sc")
                    for a in range(ND):
                        nc.tensor.matmul(sc_ps[:, :J], gt_sb[a][:, 128 * ib:128 * (ib + 1)],
                                         qxT[a][:, :J], start=(a == 0), stop=False)
                    nc.tensor.matmul(sc_ps[:, 128 * ib:J], i128b[:], mask[:],
                                     start=False, stop=True)
                    # e = exp(scores);  cs = cumsum(e);  r = cs[:, -1]
                    e_sb = wk.tile([128, S], F32, name="e_sb", tag="e_sb")
                    cs_sb = wk.tile([128, S], F32, name="cs_sb", tag="cs_sb")
                    nc.scalar.activation(e_sb[:, :J], sc_ps[:, :J], AF.Exp)
                    nc.vector.tensor_tensor_scan(cs_sb[:, :J], e_sb[:, :J], e_sb[:, :J],
                                                 0.0, OP.add, OP.bypass)
                    rr = wk.tile([128, 1], F32, name="rr", tag="rr")
                    rec = wk.tile([128, 1], F32, name="rec", tag="rec")
                    if strict and ib == 0:
                        nc.vector.tensor_scalar_max(rr[:], cs_sb[:, J - 1:J], 1e-30)
                        nc.vector.reciprocal(rec[:], rr[:])
                    else:
                        nc.vector.reciprocal(rec[:], cs_sb[:, J - 1:J])
                    # nd = cs/r - 1;  arg = theta^2 * nd * dt;  F = exp(arg)
                    nd_sb = wk.tile([128, S], BF16, name="nd_sb", tag="nd_sb")
                    arg_sb = wk.tile([128, S], BF16, name="arg_sb", tag="arg_sb")
                    f_sb = wk.tile([128, S], BF16, name="f_sb", tag="f_sb")
                    s_sb = wk.tile([128, S], BF16, name="s_sb", tag="s_sb")
                    nc.vector.tensor_scalar(nd_sb[:, :J], cs_sb[:, :J], rec[:], -1.0,
                                            OP.mult, OP.add)
                    nc.vector.scalar_tensor_tensor(arg_sb[:, :J], nd_sb[:, :J], th2b[:],
                                                   dt_sb[ib][:, :J], OP.mult, OP.mult)
                    nc.scalar.activation(f_sb[:, :J], arg_sb[:, :J], AF.Exp)
                    nc.vector.tensor_mul(s_sb[:, :J], sc_ps[:, :J], f_sb[:, :J])
                    # E2 = exp(s) (bf16) with row-sum; A = E2 / r2
                    e2_sb = wk.tile([128, S], BF16, name="e2_sb", tag="e2_sb")
                    r2 = wk.tile([128, 1], F32, name="r2", tag="r2")
                    rec2 = wk.tile([128, 1], F32, name="rec2", tag="rec2")
                    nc.scalar.activation(e2_sb[:, :J], s_sb[:, :J], AF.Exp, accum_out=r2[:])
                    if strict and ib == 0:
                        nc.vector.tensor_scalar_max(r2[:], r2[:], 1e-30)
                    nc.vector.reciprocal(rec2[:], r2[:])
                    nc.vector.tensor_scalar_mul(a_full[:, ib, :J], e2_sb[:, :J], rec2[:])
                # one transpose for the whole head: chunk q=ib*NB+jb = A_ib[:, jb-block].T
                nc.sync.dma_start_transpose(e2t[:], a_full[:].rearrange("p a j -> p (a j)"))

                    # out[e, i] += sum_j WV[j, e] A^T[j, i]
                for ec in range(ND):
                    for jb in range(NB):
                        rhs = e2t[:, NB * jb + jb:NB * NB:NB, :]
                        lhsT = wv_sb[jb // 2][:, 256 * (jb % 2) + 128 * ec:
                                              256 * (jb % 2) + 128 * (ec + 1)]
                        nc.tensor.matmul(o_ps[ec][:, 128 * jb:], lhsT, rhs,
                                         start=(h == 0 and jb == 0),
                                         stop=(h == HPC - 1 and jb == NB - 1),
                                         skip_group_check=True)

            def mha_end(st, outparts):
                for ec in range(ND):
                    nc.scalar.copy(outparts[ec][:], st["o_ps"][ec][:])

            # ---------------- phase 1: mha-q and mha-k interleaved -------------------
            sq = mha_begin("q", 0, strict=False)
            sk = mha_begin("k", 1, strict=False)
            order = [("q", 0), ("q", 1), ("k", 0), ("q", 2), ("k", 1), ("q", 3)]
            for pp, h in order:
                mha_head(sq if pp == "q" else sk, h,
                         xT_bf if pp == "q" else yT_bf,
                         xT_bf if pp == "q" else yT_bf)
            mha_end(sq, xpart)
            bncx = dram.tile([ND * 128, S], BF16, name="bncx")
            bncxo = dram.tile([ND * 128, S], BF16, name="bncxo")
            for a in range(ND):
                nc.gpsimd.dma_start(bncx[128 * a:128 * (a + 1), :], xpart[a][:])
            nc.gpsimd.collective_compute(
                "AllReduce", OP.add,
                replica_groups=[[0, 1], [2, 3], [4, 5], [6, 7]],
                ins=[bncx.opt()], outs=[bncxo.opt()])

            mha_head(sk, 2, yT_bf, yT_bf)
            mha_head(sk, 3, yT_bf, yT_bf)
            # AR1 completed during k's tail; fetch x_hat before AR2 occupies the queue
            for a in range(ND):
                nc.gpsimd.dma_start(xhT_bf[a][:], bncxo[128 * a:128 * (a + 1), :])
            mha_end(sk, ypart)
            bncy = dram.tile([ND * 128, S], BF16, name="bncy")
            bncyo = dram.tile([ND * 128, S], BF16, name="bncyo")
            for a in range(ND):
                nc.gpsimd.dma_start(bncy[128 * a:128 * (a + 1), :], ypart[a][:])
            nc.gpsimd.collective_compute(
                "AllReduce", OP.add,
                replica_groups=[[0, 1], [2, 3], [4, 5], [6, 7]],
                ins=[bncy.opt()], outs=[bncyo.opt()])
            for a in range(ND):
                nc.gpsimd.dma_start(yhT_bf[a][:], bncyo[128 * a:128 * (a + 1), :])
            if debug:
                for ec in range(ND):
                    nc.sync.dma_start(dbg_d["xhatT"][128 * ec:128 * (ec + 1), :], xhT_bf[ec][:])
                    nc.sync.dma_start(dbg_d["yhatT"][128 * ec:128 * (ec + 1), :], yhT_bf[ec][:])

            # ---------------- phase 2: mha-r (strict) -------------------------------
            sr = mha_begin("r", 2, strict=True)
            for h in range(HPC):
                mha_head(sr, h, xhT_bf, yhT_bf)
            mha_end(sr, opart)

            # ---------------- head: reduce at logit level ---------------------------
            lgo_ps = pp_sc.tile([1, S], F32, name="lgo_ps", tag="sc")
            for a in range(ND):
                nc.tensor.matmul(lgo_ps[:], wd2[:, a:a + 1], opart[a][:],
                                 start=(a == 0), stop=(a == ND - 1))
            lgo_sb = tctile(tc, [1, S], F32, name="lgo_sb")
            nc.scalar.copy(lgo_sb[:], lgo_ps[:])
            nc.gpsimd.memset(lgo_sb[:, 0:1], 0)
            bnc2 = dram.tile([1, S], F32, name="bnc2")
            bnc2o = dram.tile([1, S], F32, name="bnc2o")
            nc.gpsimd.dma_start(bnc2[:], lgo_sb[:])
            nc.gpsimd.collective_compute(
                "AllReduce", OP.add,
                replica_groups=[[0, 1], [2, 3], [4, 5], [6, 7]],
                ins=[bnc2.opt()], outs=[bnc2o.opt()])
            lgx_ps = pp_sc.tile([1, S], F32, name="lgx_ps", tag="sc")
            for a in range(ND):
                nc.tensor.matmul(lgx_ps[:], wd2[:, ND + a:ND + a + 1], xhT_bf[a][:],
                                 start=(a == 0), stop=(a == ND - 1))
            lgo2 = tctile(tc, [1, S], F32, name="lgo2")
            nc.gpsimd.dma_start(lgo2[:], bnc2o[:])
            logit = tctile(tc, [1, S], F32, name="logit")
            nc.vector.tensor_add(logit[:], lgx_ps[:], lgo2[:])
            pred = tctile(tc, [1, S], F32, name="pred")
            nc.scalar.activation(pred[:], logit[:], AF.Sigmoid, bias=bd[:])
            nc.sync.dma_start(out_d[:], pred[:])

            for free in reversed(tctile.frees[rep_free_start:]):
                free()
            del tctile.frees[rep_free_start:]

    nc.finalize()
    return nc


# ---------------------------------------------------------------------------
_NC_CACHE = {}


def _get_nc(debug=False, reps=1):
    key = (debug, reps)
    if key not in _NC_CACHE:
        _NC_CACHE[key] = build_kernel(debug, reps)
    return _NC_CACHE[key]


def _prep_core_inputs(b, g, item, timestamp, correct, Qm, c_embed, d_embed, f_embed,
                      mu_q, r_embed, Wd, bd, weights):
    f32 = np.float32
    bf = ml_dtypes.bfloat16
    it = item[b].astype(np.int64) - 1
    idx = np.where(it >= 0, it, P).astype(np.int16)
    blk = np.zeros((16, S // 16), np.int16)
    for i in range(S):
        blk[i % 16, i // 16] = idx[i]
    idx16 = np.tile(blk, (8, 1))
    ts = timestamp[b].astype(f32)
    tsa = np.stack([ts, np.ones(S, f32)])
    tsb = np.stack([np.ones(S, f32), -ts])

    i128 = np.eye(128, dtype=f32)
    r, c = np.mgrid[0:128, 0:128]
    maskns = np.where(c <= r, 0.0, NEG).astype(bf)
    masks = np.where(c < r, 0.0, NEG).astype(bf)

    def chunked(w):  # [C|D, D] f32 -> [128, chunks*D]
        n = w.shape[0] // 128
        return np.ascontiguousarray(
            w.reshape(n, 128, w.shape[1]).transpose(1, 0, 2).reshape(128, n * w.shape[1]))

    inp = {
        "qm": np.concatenate([Qm.astype(f32), np.zeros((1, C), f32)]),
        "idx16": idx16,
        "tsa": tsa,
        "tsb": tsb,
        "corr": correct[b].astype(bf).reshape(1, S),
        "i128": i128,
        "i128b": i128.astype(bf),
        "maskns": np.ascontiguousarray(maskns),
        "masks": np.ascontiguousarray(masks),
        "onescol": np.ones((128, 1), bf),
        "cembed2": chunked((mu_q * d_embed + c_embed).astype(f32)).astype(bf),
        "fembed2": chunked((mu_q * f_embed).astype(f32)).astype(bf),
        "r0": r_embed[0:1].astype(f32).astype(bf),
        "dr": (r_embed[1] - r_embed[0]).reshape(1, D).astype(f32).astype(bf),
        "th2s": np.array([[weights["q_theta"][0, 0] ** 2,
                           weights["k_theta"][0, 0] ** 2,
                           weights["r_theta"][0, 0] ** 2]], f32),
        "wd2": np.ascontiguousarray(
            np.concatenate([Wd[:D].reshape(ND, 128).T, Wd[D:].reshape(ND, 128).T],
                           axis=1)).astype(f32).astype(bf),
        "bd": bd.reshape(1, 1).astype(f32),
    }
    for p in "qkr":
        wQ, wK, wV, wO = (weights[p + "_wQ"], weights[p + "_wK"],
                          weights[p + "_wV"], weights[p + "_wO"])
        hs = range(4 * g, 4 * g + 4)
        Ms, Us = [], []
        for h in hs:
            M = (wQ[h] @ wK[h].T / np.sqrt(np.float32(D))).astype(f32)
            U = (wV[h] @ wO[h * D:(h + 1) * D]).astype(f32)
            Ms.append(chunked(M))
            Us.append(chunked(U))
        inp[p + "_m"] = np.stack(Ms).astype(bf)
        inp[p + "_wu"] = np.stack(Us).astype(bf)
    return inp


LAST_RESULTS = [None]


def kernel(item, timestamp, correct, Qm, c_embed, d_embed, f_embed, mu_q,
           r_embed, Wd, bd, q_wQ, q_wK, q_wV, q_wO, q_theta,
           k_wQ, k_wK, k_wV, k_wO, k_theta, r_wQ, r_wK, r_wV, r_wO, r_theta,
           _debug=False, _trace=False):
    weights = {
        "q_wQ": q_wQ, "q_wK": q_wK, "q_wV": q_wV, "q_wO": q_wO, "q_theta": q_theta,
        "k_wQ": k_wQ, "k_wK": k_wK, "k_wV": k_wV, "k_wO": k_wO, "k_theta": k_theta,
        "r_wQ": r_wQ, "r_wK": r_wK, "r_wV": r_wV, "r_wO": r_wO, "r_theta": r_theta,
    }
    args = (np.asarray(item), np.asarray(timestamp), np.asarray(correct),
            np.asarray(Qm), np.asarray(c_embed), np.asarray(d_embed),
            np.asarray(f_embed), np.asarray(mu_q), np.asarray(r_embed),
            np.asarray(Wd), np.asarray(bd))
    in_maps = []
    for core in range(N_CORES):
        b, g = core // 2, core % 2
        in_maps.append(_prep_core_inputs(b, g, *args, {k: np.asarray(v) for k, v in weights.items()}))
    nc = _get_nc(_debug)
    res = run_bass_kernel_spmd(nc, in_maps, core_ids=list(range(N_CORES)),
                               trace=_trace,
                               trace_cores=list(range(N_CORES)) if _trace == "all" else None)
    LAST_RESULTS[0] = res
    outs = res.results
    pred = np.zeros((B, S, 1), np.float32)
    for b in range(B):
        pred[b, :, 0] = outs[2 * b]["out"][0]
    if _debug:
        return pred, outs
    return pred

